# revision 15
# baseline (speedup 1.0000x reference)
"""VQ codebook kernel for 8 Trainium2 NeuronCores.

Problem (hardcoded): latent [64, 8, 250, 16] f32, codebook [4096, 8, 250, 16] f32,
usage_count [4096] f32.  B=64, K=4096, D=8*250*16=32000.

Strategy:
  - Shard the codebook along K across the 8 cores (512 entries/core).
  - Device (per core): one bf16 GEMM  xc[64, 512] = latent[64, D] @ cb_shard[D, 512]
    with fp32 PSUM accumulation, streamed from HBM in large contiguous DMAs.
    This is ~99% of the FLOPs and all of the memory traffic.
  - Host epilogue (cheap + exact): scores = ||c||^2 - 2*xc, take top-TOPK
    candidates per row from the bf16-accurate device scores, rescore those
    candidates exactly in fp64, then argmin / gather / scatter-add.  The
    rescore makes the returned indices exact regardless of bf16 rounding.
"""

import os
import sys

import numpy as np
import ml_dtypes

B = 64
K = 4096
D = 32000
CORES = 8
KP = K // CORES  # 512 codebook entries per core
# D is consumed in matmul chunks of 128 (PE contraction).  DMA tiles pack a
# graded number of chunks each (sum = D/128 = 250): small first tiles so the
# first matmul starts early, small last tile so the post-DMA matmul tail is
# short, big middle tiles for DMA efficiency.  DoubleRow consumes chunks in
# pairs, so every entry must be even.
CHUNKS = [int(x) for x in os.environ.get("VQ_CHUNKS", "10,20,40,50,50,50,20,6,4").split(",")]
assert sum(CHUNKS) == D // 128
TOPK = 64

_BF16 = ml_dtypes.bfloat16
_FP8 = ml_dtypes.float8_e4m3  # TRN FP8_EXP4 (bias 7, max +-240)


def _build_graph():
    import concourse.tile as tile
    from concourse import bacc, mybir

    fp8 = mybir.dt.float8e4
    nc = bacc.Bacc(None, target_bir_lowering=False)
    lat_d = nc.declare_dram_parameter("lat", [D, B], fp8, isOutput=False)
    cbt_d = nc.declare_dram_parameter("cbt", [D, KP], fp8, isOutput=False)
    out_d = nc.declare_dram_parameter("out", [B, KP], mybir.dt.float32, isOutput=True)

    nt = len(CHUNKS)
    with tile.TileContext(nc) as tc:
        with (
            tc.tile_pool(name="latp", bufs=1) as latp,
            tc.tile_pool(name="cbp", bufs=int(os.environ.get("VQ_CB_BUFS", "3"))) as cbp,
            tc.tile_pool(name="psp", bufs=1, space="PSUM") as psp,
            tc.tile_pool(name="outp", bufs=1) as outp,
        ):
            acc = psp.tile([B, KP], mybir.dt.float32)
            first = True
            off = 0  # chunk offset
            for t, c in enumerate(CHUNKS):
                d0 = off * 128
                cb_sb = cbp.tile([128, c * KP], fp8, tag="cb")
                nc.sync.dma_start(
                    cb_sb[:],
                    cbt_d[d0 : d0 + 128 * c].rearrange("(p c) n -> p (c n)", c=c),
                )
                lat_sb = latp.tile([128, c * B], fp8, tag=f"lat{t}")
                # scalar = second HWDGE ring: latent loads don't queue behind
                # the big codebook stream on the sync ring
                nc.scalar.dma_start(
                    lat_sb[:],
                    lat_d[d0 : d0 + 128 * c].rearrange("(p c) m -> p (c m)", c=c),
                )
                # Pace the PE ~15-20% faster than the DMA stream: DoubleRow
                # (2 chunks / 512-cycle matmul) would starve the PE into HAM
                # cold-clock oscillation, all-plain (1 chunk / matmul) is
                # slightly slower than the stream and becomes the critical
                # path.  Mix: ~40% of chunks through DoubleRow pairs.
                mix = os.environ.get("VQ_MIX", "dr")
                if mix == "dr":
                    n_dr = c // 2
                elif mix == "plain":
                    n_dr = 0
                else:
                    n_dr = c // 5  # DoubleRow pairs (2 chunks each)
                n_pl = c - 2 * n_dr
                last_tile = t == nt - 1
                for i in range(n_dr):
                    nc.tensor.matmul(
                        acc[:],
                        lat_sb[:, 2 * i * B : (2 * i + 2) * B].rearrange(
                            "p (two m) -> p two m", two=2
                        ),
                        cb_sb[:, 2 * i * KP : (2 * i + 2) * KP].rearrange(
                            "p (two n) -> p two n", two=2
                        ),
                        start=first,
                        stop=False,
                        perf_mode=mybir.MatmulPerfMode.DoubleRow,
                    )
                    first = False
                for i in range(2 * n_dr, c):
                    nc.tensor.matmul(
                        acc[:],
                        lat_sb[:, i * B : (i + 1) * B],
                        cb_sb[:, i * KP : (i + 1) * KP],
                        start=first,
                        stop=(last_tile and i == c - 1),
                    )
                    first = False
                off += c

            ob = outp.tile([B, KP], mybir.dt.float32)
            nc.vector.tensor_copy(ob[:], acc[:])
            nc.sync.dma_start(out_d[:], ob[:])
    nc.compile()
    return nc


def _install_ntff_hook():
    """Recreate the missing antenv.axon_hooks module so trace=True works."""
    import contextlib
    import ctypes
    import types

    so_path = "/opt/axon/libaxon_pjrt.so"
    if not os.path.exists(so_path):
        return False
    try:
        import antenv
    except ImportError:
        return False
    if "antenv.axon_hooks" in sys.modules:
        return True

    lib = ctypes.CDLL(so_path)
    if not hasattr(lib, "axon_start_nrt_profile"):
        return False
    lib.axon_start_nrt_profile.argtypes = [ctypes.POINTER(ctypes.c_int64), ctypes.c_size_t]
    lib.axon_start_nrt_profile.restype = ctypes.c_int64
    lib.axon_stop_nrt_profile.argtypes = [ctypes.c_char_p]
    lib.axon_stop_nrt_profile.restype = ctypes.c_int64

    @contextlib.contextmanager
    def _hook(output_dir, device_ids):
        import jax

        jax.devices()
        if device_ids:
            ids = (ctypes.c_int64 * len(device_ids))(*device_ids)
            rc = lib.axon_start_nrt_profile(ids, len(device_ids))
        else:
            rc = lib.axon_start_nrt_profile(None, 0)
        if rc != 0:
            raise RuntimeError(f"axon_start_nrt_profile rc={rc}")
        try:
            yield
        finally:
            n = lib.axon_stop_nrt_profile(str(output_dir).encode())
            print(f"profile: {n} ntff file(s) written to {output_dir}", file=sys.stderr)

    mod = types.ModuleType("antenv.axon_hooks")
    _h = _hook
    mod.get_axon_ntff_profile_hook = lambda: _h
    mod.set_axon_ntff_profile_hook = lambda h: None
    sys.modules["antenv.axon_hooks"] = mod
    antenv.axon_hooks = mod
    return True


_LAST_EXEC_NS = None


def kernel(latent, codebook, usage_count):
    global _LAST_EXEC_NS
    from concourse.bass_utils import run_bass_kernel_spmd

    lat_flat = np.ascontiguousarray(np.asarray(latent, dtype=np.float32).reshape(B, D))
    cb_flat = np.ascontiguousarray(np.asarray(codebook, dtype=np.float32).reshape(K, D))
    usage = np.asarray(usage_count, dtype=np.float32)

    # ---- host-side input prep (fp8 cast + per-core transpose) ----
    # Device sees latT [D, B] and cbT [D, KP]; the per-tile partition
    # grouping is expressed in the DMA access patterns, not the host layout.
    lat8 = lat_flat.astype(_FP8)
    lat_arr = np.ascontiguousarray(lat8.T)  # [D, B]
    cb8 = cb_flat.astype(_FP8)
    in_maps = []
    for g in range(CORES):
        cbt = np.ascontiguousarray(cb8[g * KP : (g + 1) * KP].T)  # [D, KP]
        in_maps.append({"lat": lat_arr, "cbt": cbt})

    # ---- device: 8-core SPMD GEMM ----
    nc = _build_graph()
    trace = bool(os.environ.get("VQ_KERNEL_TRACE"))
    kwargs = {}
    if trace and _install_ntff_hook():
        kwargs = {"trace": True}
        tdir = os.environ.get("VQ_KERNEL_TRACE_DIR")
        if tdir:
            import tempfile

            os.makedirs(tdir, exist_ok=True)
            kwargs["tmpdir"] = tempfile.mkdtemp(dir=tdir)
    res = run_bass_kernel_spmd(nc, in_maps, core_ids=list(range(CORES)), **kwargs)
    _LAST_EXEC_NS = res.exec_time_ns
    xc = np.concatenate(
        [np.asarray(res.results[g]["out"], dtype=np.float64) for g in range(CORES)], axis=1
    )  # [B, K]

    # ---- host epilogue: exact top-k rescore ----
    x2 = np.einsum("bd,bd->b", lat_flat.astype(np.float64), lat_flat.astype(np.float64))
    c2 = np.einsum("kd,kd->k", cb_flat.astype(np.float64), cb_flat.astype(np.float64))
    approx = c2[None, :] - 2.0 * xc  # [B, K]; argmin-equivalent scores
    cand = np.argpartition(approx, TOPK, axis=1)[:, :TOPK]  # [B, TOPK]

    lat64 = lat_flat.astype(np.float64)
    cb64 = cb_flat.astype(np.float64)
    indices = np.empty(B, dtype=np.int64)
    min_d2 = np.empty(B, dtype=np.float64)
    for b in range(B):
        cnd = cand[b]
        d2 = x2[b] + c2[cnd] - 2.0 * (cb64[cnd] @ lat64[b])
        a = int(np.argmin(d2))
        # jnp.argmin tie-break: first occurrence -> among equal minima pick
        # the smallest index (candidates are unordered, so sort by (d2, idx)).
        ties = np.flatnonzero(d2 == d2[a])
        indices[b] = cnd[ties].min() if len(ties) > 1 else cnd[a]
        min_d2[b] = d2[a]

    quantized = cb_flat[indices].reshape((B,) + np.asarray(latent).shape[1:]).astype(np.float32)
    min_distances = np.sqrt(np.maximum(min_d2, 0.0)).astype(np.float32)
    new_usage = usage + np.bincount(indices, minlength=K).astype(np.float32)
    return (
        quantized,
        indices.astype(np.int32),
        min_distances,
        new_usage.astype(np.float32),
    )


# revision 18
# speedup vs baseline: 1.0938x; 1.0938x over previous
"""VQ codebook kernel for 8 Trainium2 NeuronCores.

Problem (hardcoded): latent [64, 8, 250, 16] f32, codebook [4096, 8, 250, 16] f32,
usage_count [4096] f32.  B=64, K=4096, D=8*250*16=32000.

Strategy:
  - Shard the codebook along K across the 8 cores (512 entries/core).
  - Device (per core): one bf16 GEMM  xc[64, 512] = latent[64, D] @ cb_shard[D, 512]
    with fp32 PSUM accumulation, streamed from HBM in large contiguous DMAs.
    This is ~99% of the FLOPs and all of the memory traffic.
  - Host epilogue (cheap + exact): scores = ||c||^2 - 2*xc, take top-TOPK
    candidates per row from the bf16-accurate device scores, rescore those
    candidates exactly in fp64, then argmin / gather / scatter-add.  The
    rescore makes the returned indices exact regardless of bf16 rounding.
"""

import os
import sys

import numpy as np
import ml_dtypes

B = 64
K = 4096
D = 32000
CORES = 8
KP = K // CORES  # 512 codebook entries per core
# D is consumed in matmul chunks of 128 (PE contraction).  DMA tiles pack a
# graded number of chunks each (sum = D/128 = 250): small first tiles so the
# first matmul starts early, small last tile so the post-DMA matmul tail is
# short, big middle tiles for DMA efficiency.  DoubleRow consumes chunks in
# pairs, so every entry must be even.
CHUNKS = [int(x) for x in os.environ.get("VQ_CHUNKS", "10,20,40,50,50,50,20,6,4").split(",")]
assert sum(CHUNKS) == D // 128
TOPK = 64

_BF16 = ml_dtypes.bfloat16
_FP8 = ml_dtypes.float8_e4m3  # TRN FP8_EXP4 (bias 7, max +-240)


def _build_graph():
    import concourse.tile as tile
    from concourse import bacc, mybir

    fp8 = mybir.dt.float8e4
    nc = bacc.Bacc(None, target_bir_lowering=False)
    lat_d = nc.declare_dram_parameter("lat", [D, B], fp8, isOutput=False)
    cbt_d = nc.declare_dram_parameter("cbt", [D, KP], fp8, isOutput=False)
    out_d = nc.declare_dram_parameter("out", [B, KP], mybir.dt.float32, isOutput=True)

    nt = len(CHUNKS)
    with tile.TileContext(nc) as tc:
        with (
            tc.tile_pool(name="latp", bufs=1) as latp,
            tc.tile_pool(name="cbp", bufs=int(os.environ.get("VQ_CB_BUFS", "3"))) as cbp,
            tc.tile_pool(name="psp", bufs=1, space="PSUM") as psp,
            tc.tile_pool(name="outp", bufs=1) as outp,
        ):
            acc = psp.tile([B, KP], mybir.dt.float32)
            first = True
            off = 0  # chunk offset
            for t, c in enumerate(CHUNKS):
                d0 = off * 128
                cb_sb = cbp.tile([128, c * KP], fp8, tag="cb")
                nc.sync.dma_start(
                    cb_sb[:],
                    cbt_d[d0 : d0 + 128 * c].rearrange("(p c) n -> p (c n)", c=c),
                )
                lat_sb = latp.tile([128, c * B], fp8, tag=f"lat{t}")
                # scalar = second HWDGE ring: latent loads don't queue behind
                # the big codebook stream on the sync ring
                nc.scalar.dma_start(
                    lat_sb[:],
                    lat_d[d0 : d0 + 128 * c].rearrange("(p c) m -> p (c m)", c=c),
                )
                # Pace the PE ~15-20% faster than the DMA stream: DoubleRow
                # (2 chunks / 512-cycle matmul) would starve the PE into HAM
                # cold-clock oscillation, all-plain (1 chunk / matmul) is
                # slightly slower than the stream and becomes the critical
                # path.  Mix: ~40% of chunks through DoubleRow pairs.
                mix = os.environ.get("VQ_MIX", "dr")
                if mix == "dr":
                    n_dr = c // 2
                elif mix == "plain":
                    n_dr = 0
                else:
                    n_dr = c // 5  # DoubleRow pairs (2 chunks each)
                n_pl = c - 2 * n_dr
                last_tile = t == nt - 1
                for i in range(n_dr):
                    nc.tensor.matmul(
                        acc[:],
                        lat_sb[:, 2 * i * B : (2 * i + 2) * B].rearrange(
                            "p (two m) -> p two m", two=2
                        ),
                        cb_sb[:, 2 * i * KP : (2 * i + 2) * KP].rearrange(
                            "p (two n) -> p two n", two=2
                        ),
                        start=first,
                        stop=False,
                        perf_mode=mybir.MatmulPerfMode.DoubleRow,
                    )
                    first = False
                for i in range(2 * n_dr, c):
                    nc.tensor.matmul(
                        acc[:],
                        lat_sb[:, i * B : (i + 1) * B],
                        cb_sb[:, i * KP : (i + 1) * KP],
                        start=first,
                        stop=(last_tile and i == c - 1),
                    )
                    first = False
                off += c

            ob = outp.tile([B, KP], mybir.dt.float32)
            nc.vector.tensor_copy(ob[:], acc[:])
            nc.sync.dma_start(out_d[:], ob[:])
    nc.compile()
    return nc


def _build_graph_raw():
    from contextlib import ExitStack
    """Hand-scheduled variant: same dataflow as _build_graph but with manual
    semaphores and no TileContext, avoiding Tile's startup barrier and
    end-of-kernel drain/EVSEM butterfly (~6-8 us of fixed overhead)."""
    import concourse.bass as bass
    from concourse import mybir

    fp8 = mybir.dt.float8e4
    f32 = mybir.dt.float32
    nc = bass.Bass(None, target_bir_lowering=False)
    lat_d = nc.declare_dram_parameter("lat", [D, B], fp8, isOutput=False)
    cbt_d = nc.declare_dram_parameter("cbt", [D, KP], fp8, isOutput=False)
    out_d = nc.declare_dram_parameter("out", [B, KP], f32, isOutput=True)

    nt = len(CHUNKS)
    cmax = max(CHUNKS)
    offs = [sum(CHUNKS[:t]) for t in range(nt)]
    NSLOT = 3

    with (
        nc.sbuf_tensor("cb0", [128, cmax * KP], fp8) as cb0,
        nc.sbuf_tensor("cb1", [128, cmax * KP], fp8) as cb1,
        nc.sbuf_tensor("cb2", [128, cmax * KP], fp8) as cb2,
        nc.sbuf_tensor("latsb", [128, (D // 128) * B], fp8) as lat_sb,
        nc.sbuf_tensor("ob", [B, KP], f32) as ob,
        nc.psum_tensor("acc", [B, KP], f32) as acc,
        nc.semaphore("done_sem") as done_sem,
        nc.semaphore("cp_sem") as cp_sem,
        nc.semaphore("od_sem") as od_sem,
        ExitStack() as ctx,
        nc.Block() as block,
    ):
        slots = [cb0, cb1, cb2]
        # one semaphore per DMA: with several DMAs in flight on one
        # semaphore, intermediate values are ambiguous (per-engine incs from
        # different DMAs interleave), so a single-sem wait_ge(16*(t+1)) races
        cb_sems = [ctx.enter_context(nc.semaphore(f"cbs{t}")) for t in range(nt)]
        lat_sems = [ctx.enter_context(nc.semaphore(f"lats{t}")) for t in range(nt)]

        @block.sync
        def _(sync):
            for t, c in enumerate(CHUNKS):
                if t >= NSLOT:
                    # slot reuse: wait until tile t-NSLOT's matmuls are done
                    sync.wait_ge(done_sem, t - NSLOT + 1)
                d0 = offs[t] * 128
                sync.dma_start(
                    slots[t % NSLOT][:, : c * KP],
                    cbt_d[d0 : d0 + 128 * c].rearrange("(p c) n -> p (c n)", c=c),
                ).then_inc(cb_sems[t], 16)
            sync.wait_ge(cp_sem, 1)
            sync.dma_start(out_d[:], ob[:]).then_inc(od_sem, 16)
            sync.wait_ge(od_sem, 16)

        @block.scalar
        def _(scalar):
            for t, c in enumerate(CHUNKS):
                d0 = offs[t] * 128
                scalar.dma_start(
                    lat_sb[:, offs[t] * B : (offs[t] + c) * B],
                    lat_d[d0 : d0 + 128 * c].rearrange("(p c) m -> p (c m)", c=c),
                ).then_inc(lat_sems[t], 16)

        @block.tensor
        def _(tensor):
            first = True
            for t, c in enumerate(CHUNKS):
                tensor.wait_ge(cb_sems[t], 16)
                tensor.wait_ge(lat_sems[t], 16)
                cb_sb = slots[t % NSLOT]
                for i in range(c // 2):
                    j0 = (offs[t] + 2 * i) * B
                    mm = tensor.matmul(
                        acc[:],
                        lat_sb[:, j0 : j0 + 2 * B].rearrange(
                            "p (two m) -> p two m", two=2
                        ),
                        cb_sb[:, 2 * i * KP : (2 * i + 2) * KP].rearrange(
                            "p (two n) -> p two n", two=2
                        ),
                        start=first,
                        stop=(t == nt - 1 and i == c // 2 - 1),
                        perf_mode=mybir.MatmulPerfMode.DoubleRow,
                    )
                    first = False
                mm.then_inc(done_sem, 1)

        @block.vector
        def _(vector):
            vector.wait_ge(done_sem, nt)
            vector.tensor_copy(ob[:], acc[:]).then_inc(cp_sem, 1)

    return nc


def _install_ntff_hook():
    """Recreate the missing antenv.axon_hooks module so trace=True works."""
    import contextlib
    import ctypes
    import types

    so_path = "/opt/axon/libaxon_pjrt.so"
    if not os.path.exists(so_path):
        return False
    try:
        import antenv
    except ImportError:
        return False
    if "antenv.axon_hooks" in sys.modules:
        return True

    lib = ctypes.CDLL(so_path)
    if not hasattr(lib, "axon_start_nrt_profile"):
        return False
    lib.axon_start_nrt_profile.argtypes = [ctypes.POINTER(ctypes.c_int64), ctypes.c_size_t]
    lib.axon_start_nrt_profile.restype = ctypes.c_int64
    lib.axon_stop_nrt_profile.argtypes = [ctypes.c_char_p]
    lib.axon_stop_nrt_profile.restype = ctypes.c_int64

    @contextlib.contextmanager
    def _hook(output_dir, device_ids):
        import jax

        jax.devices()
        if device_ids:
            ids = (ctypes.c_int64 * len(device_ids))(*device_ids)
            rc = lib.axon_start_nrt_profile(ids, len(device_ids))
        else:
            rc = lib.axon_start_nrt_profile(None, 0)
        if rc != 0:
            raise RuntimeError(f"axon_start_nrt_profile rc={rc}")
        try:
            yield
        finally:
            n = lib.axon_stop_nrt_profile(str(output_dir).encode())
            print(f"profile: {n} ntff file(s) written to {output_dir}", file=sys.stderr)

    mod = types.ModuleType("antenv.axon_hooks")
    _h = _hook
    mod.get_axon_ntff_profile_hook = lambda: _h
    mod.set_axon_ntff_profile_hook = lambda h: None
    sys.modules["antenv.axon_hooks"] = mod
    antenv.axon_hooks = mod
    return True


_LAST_EXEC_NS = None


def kernel(latent, codebook, usage_count):
    global _LAST_EXEC_NS
    from concourse.bass_utils import run_bass_kernel_spmd

    lat_flat = np.ascontiguousarray(np.asarray(latent, dtype=np.float32).reshape(B, D))
    cb_flat = np.ascontiguousarray(np.asarray(codebook, dtype=np.float32).reshape(K, D))
    usage = np.asarray(usage_count, dtype=np.float32)

    # ---- host-side input prep (fp8 cast + per-core transpose) ----
    # Device sees latT [D, B] and cbT [D, KP]; the per-tile partition
    # grouping is expressed in the DMA access patterns, not the host layout.
    lat8 = lat_flat.astype(_FP8)
    lat_arr = np.ascontiguousarray(lat8.T)  # [D, B]
    cb8 = cb_flat.astype(_FP8)
    in_maps = []
    for g in range(CORES):
        cbt = np.ascontiguousarray(cb8[g * KP : (g + 1) * KP].T)  # [D, KP]
        in_maps.append({"lat": lat_arr, "cbt": cbt})

    # ---- device: 8-core SPMD GEMM ----
    if os.environ.get("VQ_RAW", "0") == "1":
        nc = _build_graph_raw()
    else:
        nc = _build_graph()
    trace = bool(os.environ.get("VQ_KERNEL_TRACE"))
    kwargs = {}
    if trace and _install_ntff_hook():
        kwargs = {"trace": True}
        tdir = os.environ.get("VQ_KERNEL_TRACE_DIR")
        if tdir:
            import tempfile

            os.makedirs(tdir, exist_ok=True)
            kwargs["tmpdir"] = tempfile.mkdtemp(dir=tdir)
    res = run_bass_kernel_spmd(nc, in_maps, core_ids=list(range(CORES)), **kwargs)
    _LAST_EXEC_NS = res.exec_time_ns
    xc = np.concatenate(
        [np.asarray(res.results[g]["out"], dtype=np.float64) for g in range(CORES)], axis=1
    )  # [B, K]

    # ---- host epilogue: exact top-k rescore ----
    x2 = np.einsum("bd,bd->b", lat_flat.astype(np.float64), lat_flat.astype(np.float64))
    c2 = np.einsum("kd,kd->k", cb_flat.astype(np.float64), cb_flat.astype(np.float64))
    approx = c2[None, :] - 2.0 * xc  # [B, K]; argmin-equivalent scores
    cand = np.argpartition(approx, TOPK, axis=1)[:, :TOPK]  # [B, TOPK]

    lat64 = lat_flat.astype(np.float64)
    cb64 = cb_flat.astype(np.float64)
    indices = np.empty(B, dtype=np.int64)
    min_d2 = np.empty(B, dtype=np.float64)
    for b in range(B):
        cnd = cand[b]
        d2 = x2[b] + c2[cnd] - 2.0 * (cb64[cnd] @ lat64[b])
        a = int(np.argmin(d2))
        # jnp.argmin tie-break: first occurrence -> among equal minima pick
        # the smallest index (candidates are unordered, so sort by (d2, idx)).
        ties = np.flatnonzero(d2 == d2[a])
        indices[b] = cnd[ties].min() if len(ties) > 1 else cnd[a]
        min_d2[b] = d2[a]

    quantized = cb_flat[indices].reshape((B,) + np.asarray(latent).shape[1:]).astype(np.float32)
    min_distances = np.sqrt(np.maximum(min_d2, 0.0)).astype(np.float32)
    new_usage = usage + np.bincount(indices, minlength=K).astype(np.float32)
    return (
        quantized,
        indices.astype(np.int32),
        min_distances,
        new_usage.astype(np.float32),
    )
